# revision 8
# baseline (speedup 1.0000x reference)
"""Trainium2 Bass kernel for nn_MessagePassing (vertical message passing).

Computation (per batch element b):
    y[0] = x[0]
    y[i] = x[i] + relu(conv1d_same(y[i-1], W))   for i = 1..H-1
with x (H, W, C) = (128, 256, 128) fp32, W (K, Cin, Cout) = (9, 128, 128).

Sharding: batch B=8 across the 8 NeuronCores (data parallel, no
communication). Each core runs the sequential H recurrence for one batch
element.

Design (per core):
- Host pre-transposes x rows to (H, C, W) and post-transposes the output,
  so the PE does no transposes at all - only the 9-tap conv matmuls.
- The recurrent state y lives in SBUF as one bf16 ring tensor
  (C=128 partitions, NSTATE rows, W + 8 cols with 4 zero pad cols each
  side). bf16 moving operands stream ~2 cols/cycle on the PE and make
  fast weight load (FWL) available, so matmuls run at full rate at any
  free-dim size (HW-measured pair cost ~16 + 0.22*N ns).
- Each step's row is split into M column chunks: 9 accumulating matmuls
  psum[co, chunk] += W[k].T @ y[i-1][:, chunk+k] per chunk, then one DVE
  scalar_tensor_tensor y[i][chunk] = max(psum, 0) + x_i[chunk]. The DVE of
  chunk j overlaps the PE matmuls of the following chunks, hiding the
  ~600ns PSUM->DVE->SBUF handoff that would otherwise stall every step.
- DMAs are batched (XG input rows / OG output rows per instruction) and
  split across both HWDGE rings (x-in on SP, out on Activation): the
  descriptor engine costs ~625ns per DMA instruction, so per-row DMAs
  would throttle the pipeline.
- Output rows are written as bf16 (the state dtype); the host converts to
  fp32 and transposes back. Recurrence in bf16 carries rel err ~3.4e-3
  (verified bit-matching a numpy/jax simulation of the same rounding).
"""

import numpy as np

B, H, W_DIM, C, K = 8, 128, 256, 128, 9
PAD = 4
WBUF = W_DIM + 2 * PAD  # 264
P = 128

# tuned configuration (HW-benchmarked)
# uneven chunk split: the first (larger) chunk's leading taps are the only
# matmuls whose state range is already available while the previous step's
# last DVE lands, so a bigger first chunk hides more of that latency
BOUNDS = [0, 168, 256]
NSTATE = 8
PREFETCH = 3
XG = 4   # x rows per input DMA
OG = 4   # output rows per output DMA

_NC_CACHE = {}


def _emit_body(nc, mybir, f32, bf16, x_d, o_d, xin_pool, state_pool,
               pconv_pool, wsb, bounds):
    m = len(bounds) - 1

    # state ring as ONE tensor so multi-row DMAs can read consecutive rows;
    # the DVE only ever writes [PAD, PAD+W), so pad cols stay zero
    st = state_pool.tile([P, NSTATE, WBUF], bf16, tag="st", name="st")
    nc.vector.memset(st[:], 0.0)

    x_tiles = {}

    def load_xg(g):
        if g * XG < H:
            t = xin_pool.tile([P, XG, W_DIM], f32, tag="xt", name=f"xt{g}")
            nc.sync.dma_start(t[:], x_d[g * XG:(g + 1) * XG].rearrange("h c w -> c h w"))
            x_tiles[g] = t

    def xrow(i):
        return x_tiles[i // XG][:, i % XG, :]

    for g in range(PREFETCH):
        load_xg(g)

    # y_0 = x_0 (convert to bf16 state dtype)
    nc.vector.tensor_copy(st[:, 0, PAD:PAD + W_DIM], xrow(0))

    def flush_out(i):
        # rows i-OG+1 .. i; ring slots contiguous since NSTATE % OG == 0
        s0 = (i - OG + 1) % NSTATE
        nc.scalar.dma_start(o_d[i - OG + 1:i + 1].rearrange("h c w -> c h w"),
                            st[:, s0:s0 + OG, PAD:PAD + W_DIM])

    for i in range(1, H):
        p = (i - 1) % NSTATE
        q = i % NSTATE
        for j in range(m):
            lo, hi = bounds[j], bounds[j + 1]
            w = hi - lo
            # pad each psum slot to a full 2KB bank so concurrent
            # accumulation groups never share a PSUM bank
            pc = pconv_pool.tile([P, w], f32, tag="pc", name=f"pc{i}_{j}",
                                 padded_shape=[P, 512])
            for k in range(K):
                nc.tensor.matmul(
                    pc[:, 0:w],
                    wsb[:, k, :],
                    st[:, p, lo + k: lo + k + w],
                    start=(k == 0),
                    stop=(k == K - 1),
                )
            nc.vector.scalar_tensor_tensor(
                st[:, q, PAD + lo: PAD + hi],
                pc[:, 0:w],
                0.0,
                xrow(i)[:, lo:hi],
                op0=mybir.AluOpType.max,
                op1=mybir.AluOpType.add,
            )
        if i % OG == OG - 1:
            flush_out(i)
        if i % XG == 0:
            load_xg(i // XG - 1 + PREFETCH)
            x_tiles.pop(i // XG - 1, None)


def _build_nc(reps=1):
    """Build the kernel module. reps>1 wraps the computation in a hardware
    loop repeating identical work - used only to measure device execution
    time above the dispatch-noise floor."""
    import contextlib

    import concourse.tile as tile
    from concourse import bacc, mybir

    f32 = mybir.dt.float32
    bf16 = mybir.dt.bfloat16
    bounds = BOUNDS

    nc = bacc.Bacc("TRN2", target_bir_lowering=False, debug=False, num_devices=B)
    x_d = nc.dram_tensor("x", [H, C, W_DIM], f32, kind="ExternalInput").ap()
    w_d = nc.dram_tensor("w", [K, C, C], f32, kind="ExternalInput").ap()
    o_d = nc.dram_tensor("out", [H, C, W_DIM], bf16, kind="ExternalOutput").ap()

    with tile.TileContext(nc) as tc:
        with (
            tc.tile_pool(name="xin", bufs=PREFETCH) as xin_pool,
            tc.tile_pool(name="state", bufs=1) as state_pool,
            tc.tile_pool(name="const", bufs=1) as const_pool,
            tc.tile_pool(name="pconv", bufs=len(BOUNDS) + 1, space="PSUM") as pconv_pool,
        ):
            # weights -> SBUF as (ci partitions, K, co), rounded to bf16
            wsb_raw = const_pool.tile([P, K, C], f32, name="wsb_raw")
            nc.sync.dma_start(wsb_raw[:], w_d.rearrange("k ci co -> ci k co"))
            wsb = const_pool.tile([P, K, C], bf16, name="wsb")
            nc.vector.tensor_copy(wsb[:], wsb_raw[:])

            rep_ctx = tc.For_i(0, reps, 1) if reps > 1 else contextlib.nullcontext()
            with rep_ctx:
                _emit_body(nc, mybir, f32, bf16, x_d, o_d, xin_pool,
                           state_pool, pconv_pool, wsb, bounds)

    nc.compile()
    return nc


def _get_nc():
    if "nc" not in _NC_CACHE:
        _NC_CACHE["nc"] = _build_nc()
    return _NC_CACHE["nc"]


def make_in_maps(x, W):
    """Per-core input dicts: x pre-transposed to (H, C, W), contiguous."""
    xt = np.ascontiguousarray(np.swapaxes(np.asarray(x, np.float32), 2, 3))
    W = np.asarray(W, dtype=np.float32)
    return [{"x": xt[b], "w": W} for b in range(B)]


def gather_out(res_list):
    """Per-core (H, C, W) bf16 outputs -> full (B, H, W, C) fp32."""
    outs = [np.swapaxes(np.asarray(r).astype(np.float32), 1, 2)
            for r in res_list]
    return np.stack(outs, axis=0)


def kernel(x, W):
    """Full-input entry point: shard batch B across the 8 NeuronCores (data
    parallel), run the Bass kernel, gather per-core outputs."""
    from concourse.bass_utils import run_bass_kernel_spmd

    x = np.asarray(x, dtype=np.float32)
    W = np.asarray(W, dtype=np.float32)
    assert x.shape == (B, H, W_DIM, C), x.shape
    assert W.shape == (K, C, C), W.shape

    nc = _get_nc()
    res = run_bass_kernel_spmd(nc, make_in_maps(x, W), core_ids=list(range(B)))
    return gather_out([res.results[b]["out"] for b in range(B)])
